# revision 14
# baseline (speedup 1.0000x reference)
"""CenterLoss on Trainium2 (raw Bass, SPMD over 8 NeuronCores).

Computes mean_i ||x_i - centers[label_i]||^2 (the reference clamps each
distance to [1e-12, 1e12], which never binds for this data regime).

Sharding (vocab/class-parallel, per the hint):
  - centers [100000, 512] is split row-wise into 8 shards of 12500 rows.
  - x [256, 512] and the labels are replicated to all cores.
  - Host-side sharding prep: per-core local labels = label - shard_base,
    with out-of-shard labels replaced by a huge sentinel that the
    gather's bounds check drops; a per-core f32 weight vector holds
    1/256 for in-shard rows and 0 otherwise.  Each core produces
    sum(weight_i * dist_i) — its partial of the final mean — and the
    host sums the 8 partial scalars (the unshard step).

Device program per core (identical SPMD image, different data):
  layout: batch row (t*128 + p) lives at partition p, column t.
    xt [128, 2*512] f32   <- x          (one 512 KB HWDGE load)
    lt [128, 2]     i32   <- local lbls (one HWDGE load)
    mt [128, 2]     f32   <- weights    (one HWDGE load)
    gt [128, 2*512] f32   memset 0, then 2 indirect SWDGE gathers
                          (column t <- centers_shard[lt[:, t]])
  DVE:  gt = xt - gt; per-column fused square+row-reduce
        (tensor_tensor_reduce) -> rs_t [128, 1]
  PE :  acc[1,1] += mt[:, t]^T @ rs_t   (t = 0, 1; PSUM accumulate)
        out-of-shard rows have weight 0, so whatever the bounds-checked
        gather leaves there (0 from the memset, or x-0=x after the sub)
        contributes nothing, and the memset keeps every value finite.
  ACT:  res <- acc; HWDGE stores the [1,1] scalar.

Raw Bass (not Tile) because this container's walrus build accepts only
one folded sync-wait per instruction ("Too many sync wait commands");
explicit standalone wait_ge instructions sidestep that limit.
"""

import numpy as np

import concourse.bass as bass
from concourse import mybir
from concourse.bass_utils import run_bass_kernel_spmd

NUM_CLASSES = 100000
FEAT = 512
BATCH = 256
N_CORES = 8
ROWS = NUM_CLASSES // N_CORES  # 12500 center rows per core
P = 128
NT = BATCH // P  # 2 columns per partition
OOB_SENTINEL = 2_000_000_000  # > bounds_check, still valid int32

_cache: dict = {}

# test.py reads this after calling kernel() for exec_time_ns / trace.
LAST_RESULTS = None


def _build() -> bass.Bass:
    nc = bass.Bass()
    x = nc.dram_tensor("x", [BATCH, FEAT], mybir.dt.float32, kind="ExternalInput")
    lab = nc.dram_tensor("lab", [BATCH], mybir.dt.int32, kind="ExternalInput")
    msk = nc.dram_tensor("msk", [BATCH], mybir.dt.float32, kind="ExternalInput")
    cen = nc.dram_tensor("cen", [ROWS, FEAT], mybir.dt.float32, kind="ExternalInput")
    out = nc.dram_tensor("out", [1, 1], mybir.dt.float32, kind="ExternalOutput")

    # batch row (p*NT + t) -> partition p, column t: keeps every DMA's
    # innermost dimension contiguous (row order is irrelevant to the sum)
    x_v = x.rearrange("(p t) d -> p t d", t=NT)
    lab_v = lab.rearrange("(p t) -> p t", t=NT)
    msk_v = msk.rearrange("(p t) -> p t", t=NT)

    with (
        nc.sbuf_tensor([P, NT * FEAT], mybir.dt.float32) as xt,
        nc.sbuf_tensor([P, NT * FEAT], mybir.dt.float32) as gt,
        nc.sbuf_tensor([P, NT * FEAT], mybir.dt.float32) as sq,
        nc.sbuf_tensor([P, NT], mybir.dt.int32) as lt,
        nc.sbuf_tensor([P, NT], mybir.dt.float32) as mt,
        nc.sbuf_tensor([P, NT], mybir.dt.float32) as rs,
        nc.sbuf_tensor([1, 1], mybir.dt.float32) as res,
        nc.psum_tensor([1, 1], mybir.dt.float32) as acc,
        nc.semaphore() as s_x,    # xt load done (+16)
        nc.semaphore() as s_l,    # lt load done (+16)
        nc.semaphore() as s_m,    # mt load done (+16)
        nc.semaphore() as s_ms,   # gt memset done (+1)
        nc.semaphore() as s_g,    # gathers done (+16 each)
        nc.semaphore() as s_v,    # DVE: +1 sub, +1 per ttr
        nc.semaphore() as s_mm,   # PE matmuls done (+1)
        nc.semaphore() as s_act,  # ACT copy done (+1)
        nc.semaphore() as s_out,  # final store done (+16)
        nc.Block() as block,
    ):
        gt3 = gt[:].rearrange("p (t d) -> p t d", t=NT)

        xt3 = xt[:].rearrange("p (t d) -> p t d", t=NT)

        @block.sync
        def _(sync: bass.BassEngine):
            sync.dma_start(out=xt3, in_=x_v).then_inc(s_x, 16)
            sync.dma_start(out=lt[:], in_=lab_v).then_inc(s_l, 16)
            sync.dma_start(out=mt[:], in_=msk_v).then_inc(s_m, 16)
            sync.wait_ge(s_act, 1)
            sync.dma_start(out=out[:], in_=res[:]).then_inc(s_out, 16)
            sync.wait_ge(s_out, 16)

        @block.gpsimd
        def _(gpsimd: bass.BassEngine):
            gpsimd.wait_ge(s_ms, 1)
            gpsimd.wait_ge(s_l, 16)
            for t in range(NT):
                gpsimd.indirect_dma_start(
                    out=gt3[:, t, :],
                    out_offset=None,
                    in_=cen[:],
                    in_offset=bass.IndirectOffsetOnAxis(ap=lt[:, t : t + 1], axis=0),
                    bounds_check=ROWS - 1,
                    oob_is_err=False,
                ).then_inc(s_g, 16)

        @block.vector
        def _(vector: bass.BassEngine):
            vector.memset(gt[:], 0.0).then_inc(s_ms, 1)
            vector.wait_ge(s_x, 16)
            vector.wait_ge(s_g, 32)
            vector.tensor_sub(out=gt[:], in0=xt[:], in1=gt[:])
            vector.tensor_mul(out=sq[:], in0=gt[:], in1=gt[:])
            sq3 = sq[:].rearrange("p (t d) -> p t d", t=NT)
            vector.tensor_reduce(
                out=rs[:, :],
                in_=sq3,
                op=mybir.AluOpType.add,
                axis=mybir.AxisListType.X,
            ).then_inc(s_v, 1)

        @block.tensor
        def _(tensor: bass.BassEngine):
            tensor.wait_ge(s_m, 16)
            tensor.wait_ge(s_v, 1)
            for t in range(NT):
                mm = tensor.matmul(
                    out=acc[:],
                    lhsT=mt[:, t : t + 1],
                    rhs=rs[:, t : t + 1],
                    start=(t == 0),
                    stop=(t == NT - 1),
                )
            mm.then_inc(s_mm, 1)

        @block.scalar
        def _(scalar: bass.BassEngine):
            scalar.wait_ge(s_mm, 1)
            scalar.copy(out=res[:], in_=acc[:]).then_inc(s_act, 1)

    return nc


def kernel(x: np.ndarray, label: np.ndarray, centers: np.ndarray) -> np.ndarray:
    global LAST_RESULTS
    x = np.ascontiguousarray(np.asarray(x, dtype=np.float32))
    centers = np.ascontiguousarray(np.asarray(centers, dtype=np.float32))
    lbl = np.asarray(label).astype(np.int64).ravel()
    assert x.shape == (BATCH, FEAT), x.shape
    assert centers.shape == (NUM_CLASSES, FEAT), centers.shape
    assert lbl.shape == (BATCH,), lbl.shape

    in_maps = []
    for i in range(N_CORES):
        loc = lbl - i * ROWS
        valid = (loc >= 0) & (loc < ROWS)
        loc32 = np.where(valid, loc, OOB_SENTINEL).astype(np.int32)
        wt = valid.astype(np.float32) / np.float32(BATCH)
        in_maps.append(
            {
                "x": x,
                "lab": loc32,
                "msk": wt,
                "cen": centers[i * ROWS : (i + 1) * ROWS],
            }
        )

    if "nc" not in _cache:
        _cache["nc"] = _build()
    res = run_bass_kernel_spmd(_cache["nc"], in_maps, core_ids=list(range(N_CORES)))
    LAST_RESULTS = res

    total = np.float64(0.0)
    for r in res.results:
        total += np.float64(r["out"][0, 0])
    return np.float32(total)
